# revision 30
# baseline (speedup 1.0000x reference)
"""ContiFormer-style transformer kernel for 8 Trainium2 cores.

Sharding: data-parallel over batch (B=8 -> 1 batch element per core).
All parameters replicated (tiny, d_model=64).

Device kernel design (per core, T=2048, d=64, H=4, dk=16, L=3):
  - Residual h kept transposed in SBUF: hT [64, 2048]  (d on partitions).
  - RK4 ODE evolution of k/v is linear (f(u)=u@(S-S^T)), so the whole
    evolution collapses to a per-head 16x16 matrix M = P^4, folded into
    the K/V projection weights on the host. The 1/sqrt(dk) scale is
    folded into Wq; LayerNorm scale/bias are folded into the consuming
    projection weights (y_aug carries centered*rstd, biases ride the
    ones row).
  - All matmul operands are float32r (1 PE cycle/row at moving dim
    >= 256 vs 4 for fp32); the pv product runs in bf16 (fp32r cannot
    target PSUM partition offsets, and p in [0,1] + V are bf16-safe).
  - Attention in S^T form: S^T[m,q] = sum_dk kT[dk,m] * qT[dk,q] per
    head; p = exp(S^T) (no max subtraction: logits are O(6)); the
    softmax denominator Z comes from a ones column in the token-major
    V tile, so mm2 produces [o_unnorm; Z] together.
  - LayerNorm stats via ones-matmuls on PE; rstd = exp(-0.5*ln(var+eps))
    keeps ACT on the single natural_log_exp table (no table reloads).
  - Software pipelining: the post-attention chain of chunk c (softmax
    epilogue, LN2, MLP, next layer's LN1 + q/k/v projections + V
    transpose) is emitted one step per key-block into the attention of
    chunk c+1, so PE/ACT stay busy through layer boundaries. k/v/vaug
    ping-pong between layers.
  - Layer 3 only influences the output through the LAST token: the
    device computes its LN1 + evolved k/v projections for all T (the
    heavy full-T matmuls) and streams k2/v2/h_last back; the one-query
    attention + MLP + head run on the host in float64 during gather.
"""

import numpy as np

N_LAYERS, D_MODEL, N_HEAD = 3, 64, 4
DK = D_MODEL // N_HEAD            # 16
D_INNER = 2 * D_MODEL             # 128
N_FEAT, SEQ_LEN, BATCH = 16, 2048, 8
RK4_STEPS = 4
N_CORES = 8
T = SEQ_LEN
MB = 128                          # key block (m) size
N_MB = T // MB                    # 16
QC = 512                          # query chunk for full layers
N_CH = T // QC                    # 4
VAW = 128                         # per-mb Vaug stride: 4 heads x 32 cols
EPS = 1e-5

_PROGRAM = None


def _rk4_matrix(S):
    """S: [H, dk, dk] float64 -> M = P^4 per head, P = deg-4 Taylor of exp(dt*A)."""
    A = S - np.swapaxes(S, -1, -2)
    dt = 1.0 / RK4_STEPS
    I = np.eye(DK)[None]
    dA = dt * A
    P = I + dA + dA @ dA / 2.0 + dA @ dA @ dA / 6.0 + dA @ dA @ dA @ dA / 24.0
    M = P @ P
    M = M @ M
    return M


def _blockdiag(Ms):
    out = np.zeros((D_MODEL, D_MODEL), dtype=Ms.dtype)
    for h in range(N_HEAD):
        out[h * DK:(h + 1) * DK, h * DK:(h + 1) * DK] = Ms[h]
    return out


def _spread_cols(W):
    """[.., 64] -> [.., 128] with head h's 16 cols moved to 32h..32h+16."""
    out = np.zeros(W.shape[:-1] + (128,), dtype=W.dtype)
    for h in range(N_HEAD):
        out[..., 32 * h:32 * h + DK] = W[..., DK * h:DK * (h + 1)]
    return out


def _spread_rows(W):
    out = np.zeros((128,) + W.shape[1:], dtype=W.dtype)
    for h in range(N_HEAD):
        out[32 * h:32 * h + DK] = W[DK * h:DK * (h + 1)]
    return out


def _weight_mats(inp):
    """Effective device weight matrices (float64), LN scale/bias folded in."""
    f64 = lambda a: np.asarray(a, dtype=np.float64)
    w = {}
    w["W_in_aug"] = np.vstack([f64(inp["W_in"]), f64(inp["b_in"])[None]])  # [17, 64]
    for i in range(N_LAYERS):
        s1, b1 = f64(inp["ln1_s"][i]), f64(inp["ln1_b"][i])
        s2, b2 = f64(inp["ln2_s"][i]), f64(inp["ln2_b"][i])
        Mk = _blockdiag(_rk4_matrix(f64(inp["Sk"][i])))
        Mv = _blockdiag(_rk4_matrix(f64(inp["Sv"][i])))
        # fold ln1 scale/bias into q/k/v projections
        Wk = (s1[:, None] * f64(inp["Wk"][i])) @ Mk
        bk = (b1 @ f64(inp["Wk"][i]) + f64(inp["bk"][i])) @ Mk
        Wv = (s1[:, None] * f64(inp["Wv"][i])) @ Mv
        bv = (b1 @ f64(inp["Wv"][i]) + f64(inp["bv"][i])) @ Mv
        w[f"Wk_aug_{i}"] = _spread_cols(np.vstack([Wk, bk[None]]))   # [65, 128]
        Wv_sp = _spread_cols(np.vstack([Wv, bv[None]]))              # [65, 128]
        if i == N_LAYERS - 1:
            w[f"Wv_aug_{i}"] = Wv_sp
        else:
            # token-major V projection: the moving operand is the weight, in
            # bf16 (full speed at free size 128). A 1 in the bias row at each
            # head's col 32h+DK makes the projection emit the softmax-Z ones
            # column directly, so vaug is the raw projection output.
            for h in range(N_HEAD):
                Wv_sp[D_MODEL, 32 * h + DK] = 1.0
            w[f"Wv_tok_{i}"] = Wv_sp
        if i < N_LAYERS - 1:
            Wq = (s1[:, None] * f64(inp["Wq"][i])) * 0.25
            bq = (b1 @ f64(inp["Wq"][i]) + f64(inp["bq"][i])) * 0.25
            w[f"Wq_aug_{i}"] = _spread_cols(np.vstack([Wq, bq[None]]))
            w[f"Wo_sp_{i}"] = _spread_rows(f64(inp["Wo"][i]))        # [128, 64]
            w[f"bo_row_{i}"] = f64(inp["bo"][i])[None]               # [1, 64]
            # fold ln2 scale/bias into W1
            W1 = s2[:, None] * f64(inp["W1"][i])
            bb1 = b2 @ f64(inp["W1"][i]) + f64(inp["b1"][i])
            w[f"W1_aug_{i}"] = np.vstack([W1, bb1[None]])            # [65, 128]
            w[f"W2_{i}"] = f64(inp["W2"][i])                         # [128, 64]
            w[f"b2_row_{i}"] = f64(inp["b2"][i])[None]               # [1, 64]

    Cmat = np.eye(D_MODEL) - np.full((D_MODEL, D_MODEL), 1.0 / D_MODEL)
    Dmat = np.full((D_MODEL, D_MODEL), 1.0 / D_MODEL)
    w["Cmat"] = Cmat
    w["Dmat"] = Dmat
    selZ = np.zeros((128, N_HEAD))
    for h in range(N_HEAD):
        selZ[32 * h + DK, h] = 1.0
    w["selZ"] = selZ
    B4 = np.zeros((N_HEAD, 128))
    for h in range(N_HEAD):
        B4[h, 32 * h:32 * h + DK] = 1.0
    w["B4"] = B4
    return w


# fixed packing order for the single weight DMA
_PACK_NAMES = (
    ["W_in_aug", "Cmat", "Dmat", "selZ", "B4"]
    + [f"{nm}_{i}" for i in range(2)
       for nm in ("Wq_aug", "Wk_aug", "Wv_tok", "Wo_sp", "bo_row",
                  "W1_aug", "W2", "b2_row")]
    + ["Wk_aug_2", "Wv_aug_2"]
)

_PACK_SHAPES = {
    "W_in_aug": (N_FEAT + 1, 64), "Cmat": (64, 64), "Dmat": (64, 64),
    "selZ": (128, 4), "B4": (4, 128),
    "Wk_aug_2": (65, 128), "Wv_aug_2": (65, 128),
    **{f"{nm}_{i}": shp for i in range(2) for nm, shp in (
        ("Wq_aug", (65, 128)), ("Wk_aug", (65, 128)), ("Wv_tok", (65, 128)),
        ("Wo_sp", (128, 64)), ("bo_row", (1, 64)),
        ("W1_aug", (65, 128)), ("W2", (128, 64)), ("b2_row", (1, 64)))},
}


def _pack_layout():
    lay, off = {}, 0
    for name in _PACK_NAMES:
        p, wd = _PACK_SHAPES[name]
        lay[name] = (p, off, wd)
        off += wd
    return lay, off


def _host_weights(inp):
    mats = _weight_mats(inp)
    lay, wf = _pack_layout()
    packF = np.zeros((128, wf), dtype=np.float32)
    for name, (p, off, wd) in lay.items():
        packF[0:p, off:off + wd] = mats[name]
    return {"packF": packF}


def _host_tail(inp, k2, v2, h_last):
    """Layer-2 attention (last query only) + MLP + head, float64 on host.

    k2/v2: [B, 128, T] spread layout with ODE evolution + ln1 fold already
    applied on device; h_last: [B, 64] residual entering layer 2.
    """
    f64 = lambda a: np.asarray(a, dtype=np.float64)
    i = N_LAYERS - 1

    def ln(z, s, b):
        m = z.mean(-1, keepdims=True)
        v = z.var(-1, keepdims=True)
        return (z - m) / np.sqrt(v + EPS) * s + b

    h = f64(h_last)                                       # [B, 64]
    y = ln(h, f64(inp["ln1_s"][i]), f64(inp["ln1_b"][i]))
    q = (y @ f64(inp["Wq"][i]) + f64(inp["bq"][i])) * 0.25  # [B, 64]
    qh = q.reshape(-1, N_HEAD, DK)                        # [B, H, dk]
    kh = np.stack([k2[:, 32 * h0:32 * h0 + DK, :] for h0 in range(N_HEAD)], 1)
    vh = np.stack([v2[:, 32 * h0:32 * h0 + DK, :] for h0 in range(N_HEAD)], 1)
    s = np.einsum("bhd,bhdt->bht", qh, f64(kh))           # [B, H, T]
    s -= s.max(-1, keepdims=True)
    p = np.exp(s)
    p /= p.sum(-1, keepdims=True)
    o = np.einsum("bht,bhdt->bhd", p, f64(vh)).reshape(-1, D_MODEL)
    h = h + o @ f64(inp["Wo"][i]) + f64(inp["bo"][i])
    y = ln(h, f64(inp["ln2_s"][i]), f64(inp["ln2_b"][i]))
    h = h + np.maximum(y @ f64(inp["W1"][i]) + f64(inp["b1"][i]), 0.0) \
        @ f64(inp["W2"][i]) + f64(inp["b2"][i])
    hf = ln(h, f64(inp["lnf_s"]), f64(inp["lnf_b"]))
    out = np.maximum(hf @ f64(inp["Wh1"]) + f64(inp["bh1"]), 0.0) \
        @ f64(inp["Wh2"]) + f64(inp["bh2"])
    return out.astype(np.float32)                         # [B, 1]


def _build_program():
    import concourse.bacc as bacc
    import concourse.tile as tile
    from concourse import mybir
    from concourse._compat import axon_active

    FP = mybir.dt.float32
    FR = mybir.dt.float32r
    BF = mybir.dt.bfloat16
    AF = mybir.ActivationFunctionType

    lay, WF = _pack_layout()

    nc = bacc.Bacc("TRN2", target_bir_lowering=False, debug=not axon_active())

    # All activation funcs used here (Ln, Exp) live together in the
    # natural_log_exp_and_others table, but the default chooser picks the
    # first table containing each func, forcing a ~1.3us reload per switch.
    # Hide Ln/Exp in every other table; list order (and thus the table ids)
    # is preserved and the chosen table genuinely contains all funcs.
    def _patched_table_loads():
        import bass_rust as _bass_rust
        from concourse.hw_specs import get_activation_tables
        if not any(isinstance(ins, mybir.InstActivation)
                   for b in nc.main_func.blocks for ins in b.instructions):
            return
        AFT = mybir.ActivationFunctionType
        keep = {AFT.Ln, AFT.Exp, AFT.Copy, AFT.Identity, AFT.Relu}
        tables = []
        for name, funcs in get_activation_tables(nc.m.arch).items():
            if name != "natural_log_exp_and_others":
                funcs = funcs - keep
            tables.append((name, funcs))
        _bass_rust.insert_act_table_loads(nc, tables)

    nc.insert_act_table_loads = _patched_table_loads

    d_pack = nc.dram_tensor("packF", [128, WF], FR, kind="ExternalInput").ap()
    d_xa = nc.dram_tensor("x_aug", [N_FEAT + 1, T], FR, kind="ExternalInput").ap()
    d_k2 = nc.dram_tensor("k2", [128, T], FP, kind="ExternalOutput").ap()
    d_v2 = nc.dram_tensor("v2", [128, T], FP, kind="ExternalOutput").ap()
    d_hl = nc.dram_tensor("h_last", [D_MODEL, 1], FP, kind="ExternalOutput").ap()

    with tile.TileContext(nc) as tc:
        with (
            tc.tile_pool(name="state", bufs=1) as state,
            tc.tile_pool(name="sb", bufs=3) as sb,
            tc.tile_pool(name="ppool", bufs=3) as ppool,
            tc.tile_pool(name="spool", bufs=2, space="PSUM") as spool,
            tc.tile_pool(name="opool", bufs=2, space="PSUM") as opool,
            tc.tile_pool(name="mmps", bufs=2, space="PSUM") as mmps,
        ):
            packF = state.tile([128, WF], FR, tag="packF")
            head_w = lay["Dmat"][1] + lay["Dmat"][2]   # W_in_aug + Cmat + Dmat
            nc.sync.dma_start(packF[:, 0:head_w], d_pack[:, 0:head_w])
            xa = state.tile([N_FEAT + 1, T], FR, tag="xa")
            for c in range(N_CH):
                nc.sync.dma_start(xa[:, c * QC:(c + 1) * QC],
                                  d_xa[:, c * QC:(c + 1) * QC])
            nc.sync.dma_start(packF[:, head_w:WF], d_pack[:, head_w:WF])

            def W(name):
                p, off, wd = lay[name]
                return packF[0:p, off:off + wd]

            # constants on the LN critical path first, on DVE
            hT = state.tile([D_MODEL, T], FR, tag="hT")
            y_aug = state.tile([D_MODEL + 1, T], FR, tag="y_aug")
            ones_fp = state.tile([1, T], FP, tag="ones_fp")
            nc.vector.memset(ones_fp[:], 1.0)
            nc.vector.tensor_copy(y_aug[D_MODEL:D_MODEL + 1, :], ones_fp[:])
            ones_row = state.tile([1, T], FR, tag="ones_row")
            nc.vector.tensor_copy(ones_row[:], ones_fp[:])
            eps64 = state.tile([D_MODEL, 1], FP, tag="eps64")
            nc.vector.memset(eps64[:], EPS)

            q_sp = state.tile([128, T], FR, tag="q_sp")
            r_mlp = state.tile([D_INNER, T], FR, tag="r_mlp")
            ks = [state.tile([128, T], FR, tag=f"k_sp{j}", name=f"k_sp{j}")
                  for j in range(2)]
            v2sb = state.tile([128, T], FR, tag="v2sb")
            vaugs = [state.tile([128, N_MB * VAW], BF, tag=f"vaug{j}",
                                name=f"vaug{j}") for j in range(2)]

            def matmul(out, lhsT, rhs, **kw):
                # fp32r streams 1 row/cycle at moving dim >= 256; below that
                # its cost equals fp32, so use fp32 and dodge fp32r ISA
                # restrictions at small sizes.
                if rhs.dtype == FR and rhs.free_size() < 256:
                    rhs = rhs.bitcast(FP)
                if rhs.dtype == FP and lhsT.dtype == FR:
                    lhsT = lhsT.bitcast(FP)
                nc.tensor.matmul(out, lhsT, rhs, **kw)

            def mm_tile():
                return mmps.tile([128, QC], FP, tag="mm", name="mm")

            # ---------------- chunk-level emitters ----------------
            def h0_chunk(o, w):
                ps = mm_tile()[0:D_MODEL, 0:w]
                matmul(ps, W("W_in_aug"), xa[:, o:o + w], start=True, stop=True)
                nc.scalar.activation(hT[:, o:o + w], ps, AF.Copy)

            def ln_steps(o, w):
                """centered*rstd of hT chunk -> y_aug chunk (2 emit steps)."""
                box = {}

                def a():
                    hc = mm_tile()[0:D_MODEL, 0:w]
                    matmul(hc, W("Cmat"), hT[:, o:o + w], start=True, stop=True)
                    hcs = sb.tile([D_MODEL, QC], FP, tag="hcs", name="hcs")[:, 0:w]
                    nc.vector.tensor_copy(hcs, hc)
                    sq = sb.tile([D_MODEL, QC], FR, tag="sq", name="sq")[:, 0:w]
                    nc.vector.tensor_mul(sq, hcs, hcs)
                    box["hcs"], box["sq"] = hcs, sq

                def b():
                    var = mm_tile()[0:D_MODEL, 0:w]
                    matmul(var, W("Dmat"), box["sq"], start=True, stop=True)
                    lnv = sb.tile([D_MODEL, QC], FP, tag="lnv", name="lnv")[:, 0:w]
                    nc.scalar.activation(lnv, var, AF.Ln, bias=eps64[:])
                    rstd = sb.tile([D_MODEL, QC], FP, tag="rstd", name="rstd")[:, 0:w]
                    nc.scalar.activation(rstd, lnv, AF.Exp, scale=-0.5)
                    nc.vector.tensor_mul(y_aug[0:D_MODEL, o:o + w],
                                         box["hcs"], rstd)

                return [a, b]

            def proj_step(wname, dst, o, w, dma=None, act_copy=False):
                def f():
                    p = lay[wname][2]
                    ps = mm_tile()[0:p, 0:w]
                    matmul(ps, W(wname), y_aug[:, o:o + w], start=True, stop=True)
                    if act_copy:
                        nc.scalar.activation(dst[:, o:o + w], ps, AF.Copy)
                    else:
                        nc.vector.tensor_copy(dst[:, o:o + w], ps)
                    if dma is not None:
                        nc.sync.dma_start(dma[:, o:o + w],
                                          dst[:, o:o + w].bitcast(FP))

                return f

            def vtok_half(i, vaug_t, m0):
                """token-major V projection for key-blocks m0, m0+1: the
                moving operand is the bf16 weight, so the [tokens, spread-dv]
                tile (with its Z ones column from the bias row) lands directly
                in PSUM and one plain copy fills vaug."""

                def f():
                    wv = W(f"Wv_tok_{i}")
                    for mb in (m0, m0 + 1):
                        pt = mm_tile()[:, 0:VAW]
                        matmul(pt, y_aug[:, mb * MB:(mb + 1) * MB], wv,
                               start=True, stop=True)
                        nc.vector.tensor_copy(
                            vaug_t[:, mb * VAW:(mb + 1) * VAW], pt)

                return f

            def mlp_steps(i, o, w):
                def a():
                    ps1 = mm_tile()[0:D_INNER, 0:w]
                    matmul(ps1, W(f"W1_aug_{i}"), y_aug[:, o:o + w],
                           start=True, stop=True)
                    nc.vector.tensor_scalar(r_mlp[:, o:o + w], ps1, 0.0, None,
                                            op0=mybir.AluOpType.max)

                def b():
                    ps2 = mm_tile()[0:D_MODEL, 0:w]
                    matmul(ps2, W(f"W2_{i}"), r_mlp[:, o:o + w],
                           start=True, stop=False)
                    matmul(ps2, W(f"b2_row_{i}"), ones_row[:, 0:w],
                           start=False, stop=True)
                    dst = hT[:, o:o + w]
                    nc.vector.tensor_add(dst, dst, ps2)

                return [a, b]

            def ep_steps(i, ops, out_lo, cw):
                """softmax epilogue: normalize, Wo project, add to residual."""
                box = {}

                def e1():
                    osb = sb.tile([128, QC], FR, tag="osb", name="osb")[:, 0:cw]
                    nc.vector.tensor_copy(osb, ops)
                    box["osb"] = osb

                def e2():
                    zc = mm_tile()[0:N_HEAD, 0:cw]
                    matmul(zc, W("selZ"), box["osb"], start=True, stop=True)
                    zi = sb.tile([N_HEAD, QC], FR, tag="zi", name="zi")[:, 0:cw]
                    with nc.allow_low_precision(reason="fp32r rounding of 1/Z"):
                        nc.vector.reciprocal(zi, zc)
                    box["zi"] = zi

                def e3():
                    zb = mm_tile()[:, 0:cw]
                    matmul(zb, W("B4"), box["zi"], start=True, stop=True)
                    osc = sb.tile([128, QC], FR, tag="osc", name="osc")[:, 0:cw]
                    nc.vector.tensor_mul(osc, box["osb"], zb)
                    box["osc"] = osc

                def e4():
                    hd = mm_tile()[0:D_MODEL, 0:cw]
                    matmul(hd, W(f"Wo_sp_{i}"), box["osc"], start=True, stop=False)
                    matmul(hd, W(f"bo_row_{i}"), ones_row[:, 0:cw],
                           start=False, stop=True)
                    dst = hT[:, out_lo:out_lo + cw]
                    nc.vector.tensor_add(dst, dst, hd)

                return [e1, e2, e3, e4]

            def att_chunk(kbuf, vaug_t, qlo, cw, steps):
                """attention for queries [qlo, qlo+cw); pops one pipelined
                step-slot after each key-block. Returns the ops PSUM tile."""
                ops = opool.tile([128, QC], FP, tag="o_ps", name="o_ps")[:, 0:cw]
                for mb in range(N_MB):
                    for pair in range(2):
                        st = spool.tile([128, 2 * QC], FP, tag="s_ps", name="s_ps")
                        for hh in range(2):
                            h = 2 * pair + hh
                            matmul(st[:, hh * QC:hh * QC + cw],
                                   kbuf[32 * h:32 * h + DK, :].rearrange(
                                       "p (m c) -> p m c", m=N_MB)[:, mb],
                                   q_sp[32 * h:32 * h + DK, qlo:qlo + cw],
                                   start=True, stop=True,
                                   tile_position=(32 * h, 0))
                        pt = ppool.tile([128, 2 * QC], BF, tag="p_sb", name="p_sb")
                        nc.scalar.activation(pt[:], st[:], AF.Exp)
                        for hh in range(2):
                            h = 2 * pair + hh
                            matmul(ops[32 * h:32 * h + 32, :],
                                   vaug_t[:, mb * VAW + h * 32:
                                          mb * VAW + (h + 1) * 32],
                                   pt[:, hh * cw:(hh + 1) * cw],
                                   start=(mb == 0), stop=(mb == N_MB - 1),
                                   tile_position=(0, 32 * h),
                                   skip_group_check=True)
                    if steps:
                        for f in steps.pop(0):
                            f()
                while steps:
                    for f in steps.pop(0):
                        f()
                return ops

            def post_slots(i, c, ops):
                """pipelined steps after attention chunk c of layer i:
                epilogue, LN2+MLP, then next layer's LN1/projections."""
                o, w = c * QC, QC
                slots = [[s] for s in ep_steps(i, ops, o, w)]       # 4
                slots += [[s] for s in ln_steps(o, w)]              # LN2: 2
                slots += [[s] for s in mlp_steps(i, o, w)]          # 2
                j = i + 1
                slots += [[s] for s in ln_steps(o, w)]              # LN1': 2
                if j < N_LAYERS - 1:
                    # Packed so a boundary chunk's products land before their
                    # first consumer in the next attention chunk's emission
                    # order: k' by slot 10 (scores of key-block 12 fire at
                    # slot 12), vaug halves by slots 11/12.
                    nb = j % 2
                    slots += [[proj_step(f"Wk_aug_{j}", ks[nb], o, w)]]
                    slots += [[vtok_half(j, vaugs[nb], 4 * c)]]
                    slots += [[vtok_half(j, vaugs[nb], 4 * c + 2)]]
                    slots += [[proj_step(f"Wq_aug_{j}", q_sp, o, w)]]
                else:
                    # last layer: stream evolved k/v straight to HBM for the
                    # host-side single-query tail
                    slots += [[proj_step(f"Wk_aug_{j}", ks[0], o, w, dma=d_k2)]]
                    slots += [[proj_step(f"Wv_aug_{j}", v2sb, o, w, dma=d_v2)]]
                    if c == N_CH - 1:
                        def hl():
                            nc.sync.dma_start(d_hl[:], hT[:, T - 1:T].bitcast(FP))
                        slots += [[hl]]
                return slots

            # ---------------- program ----------------
            # input projection + layer-0 LN/projections (no attention to
            # hide behind yet; chunks pipeline across engines)
            # only chunk 0's LN/projections gate the first attention chunk;
            # chunks 1-3 ride its key-block slots (k(c) must be emitted
            # before the key blocks that read it: k(c1) by slot 3, k(c2) by
            # slot 7, k(c3) by slot 11; vtok halves one slot later)
            for c in range(N_CH):
                h0_chunk(c * QC, QC)
            for s in ln_steps(0, QC):
                s()
            proj_step("Wk_aug_0", ks[0], 0, QC, act_copy=True)()
            vtok_half(0, vaugs[0], 0)()
            vtok_half(0, vaugs[0], 2)()
            proj_step("Wq_aug_0", q_sp, 0, QC, act_copy=True)()

            pending = []
            for c in range(1, N_CH):
                o = c * QC
                la, lb = ln_steps(o, QC)
                pending += [[la], [lb],
                            [proj_step("Wk_aug_0", ks[0], o, QC),
                             vtok_half(0, vaugs[0], 4 * c)],
                            [vtok_half(0, vaugs[0], 4 * c + 2)]]
            pending += [[proj_step("Wq_aug_0", q_sp, QC, QC)],
                        [proj_step("Wq_aug_0", q_sp, 2 * QC, QC)],
                        [proj_step("Wq_aug_0", q_sp, 3 * QC, QC)]]
            for i in range(2):
                for c in range(N_CH):
                    ops = att_chunk(ks[i % 2], vaugs[i % 2], c * QC, QC, pending)
                    pending = post_slots(i, c, ops)
            # drain the last chunk's chain (k2/v2/h_last DMA-out tail)
            while pending:
                for f in pending.pop(0):
                    f()

    nc.compile()
    return nc


def _get_program(weights=None):
    global _PROGRAM
    if _PROGRAM is None:
        _PROGRAM = _build_program()
    return _PROGRAM


def kernel(**inputs):
    weights = _host_weights(inputs)
    nc = _get_program()

    x = np.asarray(inputs["x"], dtype=np.float32)            # [8, 16, 2048]
    in_maps = []
    for b in range(N_CORES):
        xa = np.concatenate([x[b], np.ones((1, T), np.float32)], axis=0)
        in_maps.append({"x_aug": np.ascontiguousarray(xa), **weights})

    from concourse.bass_utils import run_bass_kernel_spmd
    res = run_bass_kernel_spmd(nc, in_maps, list(range(N_CORES)))
    k2 = np.stack([res.results[b]["k2"] for b in range(N_CORES)], axis=0)
    v2 = np.stack([res.results[b]["v2"] for b in range(N_CORES)], axis=0)
    h_last = np.stack([res.results[b]["h_last"].reshape(-1)
                       for b in range(N_CORES)], axis=0)
    return _host_tail(inputs, k2, v2, h_last)
